# revision 14
# baseline (speedup 1.0000x reference)
"""Trainium2 Bass kernel for a 2-layer linear RNN (identity state transition).

Math: the reference computes, per layer l, h = cumsum_t(h @ W_l^T) and then
outputs = h @ W_out^T.  Cumsum along time commutes with the (time-independent)
feature matmuls, so with Wa = W1 @ W0 and Wb = W_out @ Wa:

    hidden  = cumsum_t(cumsum_t(x)) @ Wa^T
    outputs = cumsum_t(cumsum_t(x)) @ Wb^T

The double cumsum y = C^2 x has closed form y[t] = sum_{s<=t} (t-s+1) x[s].
Blockwise (128-step blocks) it is computed on the PE in two parts:

  1. LOCAL: yT_chunk = matmul(lhsT=x_chunk, rhs=T2U) with the constant
     triangular T2U[s,t'] = (t'-s+1 for s<=t') gives [feature, time] chunks
     with no explicit PE transposes; yT is exactly the operand layout the two
     weight matmuls need as lhsT.  The local projections ph/po = yT.T @ W are
     carry-free.
  2. CARRY (applied POST-projection): the cross-block contribution of block q
     to block k at local time tau is (128*D + tau - 127)*ba(q) + r127(q) with
     D = k-q, where ba(q) (block sum of projected inputs) and r127(q) (last
     row) are linear functions of rows 126/127 of the LOCAL projection of
     block q.  Tiny per-block row extractions (GpSimd) append
     [ba; r127] to a persistent Rstack[64, 512]; the whole carry for block k
     is then ONE matmul per output: maskT_k[0:2k].T @ Rstack[0:2k] with a
     host-precomputed constant mask table.  (ba is stored explicitly --
     forming r127-r126 inside the matmul would amplify fp16 quantization by
     |r127|/|ba| ~ 70x.)

This removes all per-block carry-state matmuls from the v2 design.  A
software pipeline (projection runs one block behind the local cumsum,
correction one further behind) keeps the PE dense and hides the PSUM->SBUF
cast latencies.

Dtype strategy: everything on-device is float16 (inputs pre-scaled by 1/64 on
the host so the double-cumsum magnitudes stay inside fp16 range; outputs are
scaled back by 64 on the host).  fp16 matmuls run the PE at the full 2.4 GHz
warm clock with fast weight load, stream 1 column/cycle, and halve DMA and
on-chip copy traffic vs fp32.  PSUM accumulation stays fp32.  All constant
coefficient tables (integers <= 4096) are exact-to-half-ulp in fp16.

Sharding: data-parallel over batch, 2 of 16 batch elements per core, weights
replicated.
"""

import numpy as np

import concourse.bass as bass
import concourse.bacc as bacc
import concourse.mybir as mybir
from concourse.tile import TileContext
from concourse.bass_utils import run_bass_kernel_spmd

P = 128          # partitions / time-block size
H = 512          # hidden/input/output feature dim
T = 4096         # sequence length
B = 16           # batch
NCORES = 8
BPC = B // NCORES            # batch elements per core = 2
NSUB = 4                     # 128-step sub-tiles per super-tile
SUPER = P * NSUB             # 512 timesteps per DMA super-tile
NGB = T // P                 # 128-step blocks per batch element = 32

F32 = mybir.dt.float32
F16 = mybir.dt.float16

SCALE = 1.0 / 64.0           # host pre-scale keeping fp16 in range

# column offsets inside the packed fp16 constant block
C_WA = 0
C_WB = C_WA + 4 * H          # 2048
C_T2U = C_WB + 4 * H         # 4096
C_MASK = C_T2U + P           # 4224: 32 carry-mask tiles [64 x 128] side by side
C_TOT = C_MASK + NGB * P     # 8320


def build_nc(bpc: int = BPC, t_len: int = T) -> bass.Bass:
    ngb = t_len // P         # 128-step blocks per batch element
    nc = bacc.Bacc(None, target_bir_lowering=False)

    x_d = nc.dram_tensor("x", [bpc * t_len, H], F16, kind="ExternalInput")
    cpack_d = nc.dram_tensor("cpack", [P, C_TOT], F16, kind="ExternalInput")
    out_d = nc.dram_tensor("outputs", [bpc * t_len, H], F16, kind="ExternalOutput")
    hid_d = nc.dram_tensor("hidden", [bpc * t_len, H], F16, kind="ExternalOutput")

    with TileContext(nc) as tc:
        with (
            tc.tile_pool(name="consts", bufs=1) as cpool,
            tc.tile_pool(name="xs", bufs=3) as xpool,
            tc.tile_pool(name="staged", bufs=3) as stpool,
            tc.tile_pool(name="ytsb", bufs=3) as ytpool,
            tc.tile_pool(name="rstack", bufs=2) as rpool,
            tc.tile_pool(name="rscr", bufs=4) as scrpool,
            tc.tile_pool(name="psyt", bufs=2, space="PSUM") as psyt,
            tc.tile_pool(name="pso", bufs=2, space="PSUM") as pso,
        ):
            cpack = cpool.tile([P, C_TOT], F16)
            nc.sync.dma_start(out=cpack[:], in_=cpack_d[:])

            wa_sb = cpack[:, C_WA : C_WA + 4 * H]
            wb_sb = cpack[:, C_WB : C_WB + 4 * H]
            t2u_sb = cpack[:, C_T2U : C_T2U + P]

            for b in range(bpc):
                Ra = rpool.tile([2 * NGB, H], F16, tag="Ra")
                Rb = rpool.tile([2 * NGB, H], F16, tag="Rb")
                xsup = {}
                h2sup = {}
                outsup = {}
                pyts = {}
                yts = {}
                phs = {}
                pos = {}
                for k in range(ngb + 2):
                    # ---- stage 0: DMA in super-tile
                    if k < ngb and k % NSUB == 0:
                        g = k // NSUB
                        base = b * t_len + g * SUPER
                        xs = xpool.tile([P, NSUB, H], F16, name="xs")
                        nc.sync.dma_start(
                            out=xs[:],
                            in_=x_d[base : base + SUPER, :].rearrange(
                                "(n p) h -> p n h", p=P
                            ),
                        )
                        xsup[g] = xs
                        h2sup[g] = stpool.tile([P, NSUB, H], F16, tag="h2s", name="h2s")
                        outsup[g] = stpool.tile([P, NSUB, H], F16, tag="outs", name="outs")

                    # ---- stage 1: local double cumsum for block k
                    if k < ngb:
                        x_t = xsup[k // NSUB][:, k % NSUB, :]
                        pyt = psyt.tile([P, H], F32, name="pyt")
                        for c in range(4):
                            nc.tensor.matmul(
                                pyt[:, c * P : (c + 1) * P],
                                x_t[:, c * P : (c + 1) * P],
                                t2u_sb,
                                start=True, stop=True,
                            )
                        pyts[k] = pyt

                    # ---- stage 3 (two blocks behind): carry corr + copies
                    j = k - 2
                    if j >= 0:
                        ph, po = phs.pop(j), pos.pop(j)
                        # rows [0:2j] apply the cross-block carry; rows
                        # [2j:2j+2] repair this block's own row 127 (the
                        # projection emitted ba there, not r127)
                        mask = cpack[0 : 2 * j + 2, C_MASK + j * P : C_MASK + (j + 1) * P]
                        nc.tensor.matmul(
                            ph[:], mask, Ra[0 : 2 * j + 2, :],
                            start=False, stop=True, skip_group_check=True,
                        )
                        nc.tensor.matmul(
                            po[:], mask, Rb[0 : 2 * j + 2, :],
                            start=False, stop=True, skip_group_check=True,
                        )
                        g, n = divmod(j, NSUB)
                        nc.scalar.copy(h2sup[g][:, n, :], ph[:])
                        nc.scalar.copy(outsup[g][:, n, :], po[:])
                        if n == NSUB - 1:
                            base = b * t_len + g * SUPER
                            nc.sync.dma_start(
                                out=hid_d[base : base + SUPER, :].rearrange(
                                    "(n p) h -> p n h", p=P
                                ),
                                in_=h2sup.pop(g)[:],
                            )
                            nc.sync.dma_start(
                                out=out_d[base : base + SUPER, :].rearrange(
                                    "(n p) h -> p n h", p=P
                                ),
                                in_=outsup.pop(g)[:],
                            )
                            xsup.pop(g, None)

                    # ---- stage 1b: cast yT to fp16 for the projections
                    if k < ngb:
                        yt = ytpool.tile([P, H], F16, name="yt")
                        nc.vector.tensor_copy(yt[:], pyts.pop(k)[:])
                        yts[k] = yt

                    # ---- stage 2 (one block behind): local projections
                    i = k - 1
                    if 0 <= i < ngb:
                        yt = yts.pop(i)
                        ph = pso.tile([P, H], F32, tag="ph", name="ph")
                        po = pso.tile([P, H], F32, tag="po", name="po")
                        for c in range(4):
                            lhs = yt[:, c * P : (c + 1) * P]
                            nc.tensor.matmul(
                                ph[:], lhs, wa_sb[:, c * H : (c + 1) * H],
                                start=(c == 0), stop=(c == 3),
                            )
                            nc.tensor.matmul(
                                po[:], lhs, wb_sb[:, c * H : (c + 1) * H],
                                start=(c == 0), stop=(c == 3),
                            )
                        phs[i], pos[i] = ph, po
                        # row extraction: T2U's last column is all-ones, so the
                        # projection's psum row 127 holds ba (the projected
                        # block sum, computed WITHOUT cancellation); row 126 is
                        # the true r126.  Store (r126, ba) raw -- the mask
                        # algebra reconstructs r127 = r126 + ba.  One two-lane
                        # cast-copy per output (GpSimd can't touch PSUM; engine
                        # SBUF writes must start at partition 0/32/64/96), then
                        # a tiny SBUF->SBUF DMA scatters into Rstack rows
                        # 2i:2i+2 (DMA has no partition alignment limits).
                        # (engine PSUM reads must also start at a quadrant
                        # boundary, so copy the whole last quadrant -- the 32
                        # lanes run in parallel, same duration as 2 rows)
                        for R, pp, tg, eng in (
                            (Ra, ph, "sa", nc.vector),
                            (Rb, po, "sb", nc.scalar),
                        ):
                            s2 = scrpool.tile([32, H], F16, tag=tg, name="s2")
                            if eng is nc.vector:
                                eng.tensor_copy(s2[:, :], pp[96:128, :])
                            else:
                                eng.copy(s2[:, :], pp[96:128, :])
                            nc.sync.dma_start(
                                out=R[2 * i : 2 * i + 2, :], in_=s2[30:32, :]
                            )
    if not nc.is_finalized():
        nc.finalize()
    return nc


def make_consts(W_ih: np.ndarray, W_out: np.ndarray) -> dict[str, np.ndarray]:
    W0 = W_ih[0].astype(np.float64)
    W1 = W_ih[1].astype(np.float64)
    Wa64 = W1 @ W0
    Wb64 = W_out.astype(np.float64) @ Wa64

    # [i, o] chunked along i into 4 partition groups -> [128, 4*512]
    def pack_w(w64):
        wT = w64.T.astype(np.float16)  # [i, o]
        return np.ascontiguousarray(
            wT.reshape(4, P, H).transpose(1, 0, 2).reshape(P, 4 * H)
        )

    tau = np.arange(P, dtype=np.float32)
    s_idx = tau[:, None]
    t_idx = tau[None, :]

    cpack = np.zeros((P, C_TOT), dtype=np.float32)
    cpack[:, C_WA : C_WA + 4 * H] = pack_w(Wa64)
    cpack[:, C_WB : C_WB + 4 * H] = pack_w(Wb64)
    t2u = np.where(t_idx >= s_idx, t_idx - s_idx + 1.0, 0.0)
    # last column emits the raw block sum instead of y_loc[:,127] so the
    # projection produces ba (no catastrophic cancellation); the mask's
    # self-fix rows repair output row 127 afterwards
    t2u[:, P - 1] = 1.0
    cpack[:, C_T2U : C_T2U + P] = t2u
    # carry mask tiles.  Rstack rows: 2q = r126(q), 2q+1 = ba(q); the true
    # last row is r127 = r126 + ba, so
    #   carry(k, tau) = sum_{q<k} (128*(k-q)+tau-127)*ba(q) + r127(q)
    #                 = sum_{q<k} (128*(k-q)+tau-126)*ba(q) + r126(q)
    # plus a self-fix row for block k: output row 127 currently holds ba(k)
    # and must become r127(k) -> add r126(k) at tau=127 only.
    for k in range(NGB):
        col = C_MASK + k * P
        for q in range(k):
            d = float(k - q)
            cpack[2 * q, col : col + P] = 1.0
            cpack[2 * q + 1, col : col + P] = 128.0 * d + tau - 126.0
        cpack[2 * k, col + P - 1] = 1.0
    return {"cpack": cpack.astype(np.float16)}


def make_in_maps(x: np.ndarray, W_ih: np.ndarray, W_out: np.ndarray):
    consts = make_consts(np.asarray(W_ih, np.float32), np.asarray(W_out, np.float32))
    xs = (np.asarray(x, np.float32) * SCALE).astype(np.float16)
    in_maps = []
    for core in range(NCORES):
        shard = np.ascontiguousarray(
            xs[core * BPC : (core + 1) * BPC].reshape(BPC * T, H)
        )
        in_maps.append({"x": shard, **consts})
    return in_maps


def gather_outputs(results):
    outs = np.concatenate(
        [r["outputs"].reshape(BPC, T, H).astype(np.float32) for r in results],
        axis=0,
    ) * (1.0 / SCALE)
    hids = np.concatenate(
        [r["hidden"].reshape(BPC, T, H).astype(np.float32) for r in results],
        axis=0,
    ) * (1.0 / SCALE)
    return outs, hids


def kernel(x: np.ndarray, W_ih: np.ndarray, W_out: np.ndarray):
    nc = build_nc()
    in_maps = make_in_maps(x, W_ih, W_out)
    res = run_bass_kernel_spmd(nc, in_maps, core_ids=list(range(NCORES)))
    return gather_outputs(res.results)


# revision 20
# speedup vs baseline: 1.0593x; 1.0593x over previous
"""Trainium2 Bass kernel for a 2-layer linear RNN (identity state transition).

Math: the reference computes, per layer l, h = cumsum_t(h @ W_l^T) and then
outputs = h @ W_out^T.  Cumsum along time commutes with the (time-independent)
feature matmuls, so with Wa = W1 @ W0 and Wb = W_out @ Wa:

    hidden  = cumsum_t(cumsum_t(x)) @ Wa^T
    outputs = cumsum_t(cumsum_t(x)) @ Wb^T

The double cumsum y = C^2 x has closed form y[t] = sum_{s<=t} (t-s+1) x[s].
Blockwise (128-step blocks) it is computed on the PE in two parts:

  1. LOCAL: yT_chunk = matmul(lhsT=x_chunk, rhs=T2U) with the constant
     triangular T2U[s,t'] = (t'-s+1 for s<=t') gives [feature, time] chunks
     with no explicit PE transposes; yT is exactly the operand layout the two
     weight matmuls need as lhsT.  The local projections ph/po = yT.T @ W are
     carry-free.
  2. CARRY (applied POST-projection): the cross-block contribution of block q
     to block k at local time tau is (128*D + tau - 127)*ba(q) + r127(q) with
     D = k-q, where ba(q) (block sum of projected inputs) and r127(q) (last
     row) are linear functions of rows 126/127 of the LOCAL projection of
     block q.  Tiny per-block row extractions (GpSimd) append
     [ba; r127] to a persistent Rstack[64, 512]; the whole carry for block k
     is then ONE matmul per output: maskT_k[0:2k].T @ Rstack[0:2k] with a
     host-precomputed constant mask table.  (ba is stored explicitly --
     forming r127-r126 inside the matmul would amplify fp16 quantization by
     |r127|/|ba| ~ 70x.)

This removes all per-block carry-state matmuls from the v2 design.  A
software pipeline (projection runs one block behind the local cumsum,
correction one further behind) keeps the PE dense and hides the PSUM->SBUF
cast latencies.

Dtype strategy: everything on-device is float16 (inputs pre-scaled by 1/64 on
the host so the double-cumsum magnitudes stay inside fp16 range; outputs are
scaled back by 64 on the host).  fp16 matmuls run the PE at the full 2.4 GHz
warm clock with fast weight load, stream 1 column/cycle, and halve DMA and
on-chip copy traffic vs fp32.  PSUM accumulation stays fp32.  All constant
coefficient tables (integers <= 4096) are exact-to-half-ulp in fp16.

Sharding: data-parallel over batch, 2 of 16 batch elements per core, weights
replicated.
"""

import numpy as np

import concourse.bass as bass
import concourse.bacc as bacc
import concourse.mybir as mybir
from concourse.tile import TileContext
from concourse.bass_utils import run_bass_kernel_spmd

P = 128          # partitions / time-block size
H = 512          # hidden/input/output feature dim
T = 4096         # sequence length
B = 16           # batch
NCORES = 8
BPC = B // NCORES            # batch elements per core = 2
NSUB = 4                     # 128-step sub-tiles per super-tile
SUPER = P * NSUB             # 512 timesteps per DMA super-tile
NGB = T // P                 # 128-step blocks per batch element = 32

F32 = mybir.dt.float32
F16 = mybir.dt.float16

SCALE = 1.0 / 64.0           # host pre-scale keeping fp16 in range

# column offsets inside the packed fp16 constant block
C_WA = 0
C_WB = C_WA + 4 * H          # 2048
C_T2U = C_WB + 4 * H         # 4096
C_MASK = C_T2U + P           # 4224: 32 carry-mask tiles [64 x 128] side by side
C_TOT = C_MASK + NGB * P     # 8320


def build_nc(bpc: int = BPC, t_len: int = T) -> bass.Bass:
    ngb = t_len // P         # 128-step blocks per batch element
    nc = bacc.Bacc(None, target_bir_lowering=False)

    x_d = nc.dram_tensor("x", [bpc * t_len, H], F16, kind="ExternalInput")
    cpack_d = nc.dram_tensor("cpack", [P, C_TOT], F16, kind="ExternalInput")
    out_d = nc.dram_tensor("outputs", [bpc * t_len, H], F16, kind="ExternalOutput")
    hid_d = nc.dram_tensor("hidden", [bpc * t_len, H], F16, kind="ExternalOutput")

    with TileContext(nc) as tc:
        with (
            tc.tile_pool(name="consts", bufs=1) as cpool,
            tc.tile_pool(name="xs", bufs=3) as xpool,
            tc.tile_pool(name="staged", bufs=3) as stpool,
            tc.tile_pool(name="ytsb", bufs=3) as ytpool,
            tc.tile_pool(name="rstack", bufs=2) as rpool,
            tc.tile_pool(name="rscr", bufs=4) as scrpool,
            tc.tile_pool(name="psyt", bufs=2, space="PSUM") as psyt,
            tc.tile_pool(name="pso", bufs=3, space="PSUM") as pso,
        ):
            cpack = cpool.tile([P, C_TOT], F16)
            nc.sync.dma_start(out=cpack[:], in_=cpack_d[:])

            wa_sb = cpack[:, C_WA : C_WA + 4 * H]
            wb_sb = cpack[:, C_WB : C_WB + 4 * H]
            t2u_sb = cpack[:, C_T2U : C_T2U + P]

            for b in range(bpc):
                # rows 2q = r126(q), 2q+1 = ba(q); cols [0:H] for the hidden
                # projection, [H:2H] for the output projection.  Zeroed so the
                # constant-K=128 corr matmuls can read not-yet-written rows
                # (their mask coefficients are zero, but NaN*0 != 0).
                Rab = rpool.tile([P, 2 * H], F16, tag="Rab")
                nc.gpsimd.memset(Rab[:], 0.0)
                xsup = {}
                h2sup = {}
                outsup = {}
                pyts = {}
                yts = {}
                phs = {}
                pos = {}
                for k in range(ngb + 3):
                    # ---- stage 0: DMA in super-tile
                    if k < ngb and k % NSUB == 0:
                        g = k // NSUB
                        base = b * t_len + g * SUPER
                        xs = xpool.tile([P, NSUB, H], F16, name="xs")
                        nc.sync.dma_start(
                            out=xs[:],
                            in_=x_d[base : base + SUPER, :].rearrange(
                                "(n p) h -> p n h", p=P
                            ),
                        )
                        xsup[g] = xs
                        h2sup[g] = stpool.tile([P, NSUB, H], F16, tag="h2s", name="h2s")
                        outsup[g] = stpool.tile([P, NSUB, H], F16, tag="outs", name="outs")

                    # ---- stage 1: local double cumsum for block k
                    if k < ngb:
                        x_t = xsup[k // NSUB][:, k % NSUB, :]
                        pyt = psyt.tile([P, H], F32, name="pyt")
                        for c in range(4):
                            nc.tensor.matmul(
                                pyt[:, c * P : (c + 1) * P],
                                x_t[:, c * P : (c + 1) * P],
                                t2u_sb,
                                start=True, stop=True,
                            )
                        pyts[k] = pyt

                    # ---- stage 3 (three blocks behind): carry corr + copies
                    j = k - 3
                    if j >= 0:
                        ph, po = phs.pop(j), pos.pop(j)
                        # rows [0:2j] apply the cross-block carry; rows
                        # [2j:2j+2] repair this block's own row 127 (the
                        # projection emitted ba there, not r127).  K is padded
                        # to the full 128 (zero mask rows) to stay on the
                        # full-array matmul path.
                        mask = cpack[:, C_MASK + j * P : C_MASK + (j + 1) * P]
                        nc.tensor.matmul(
                            ph[:], mask, Rab[:, 0:H],
                            start=False, stop=True, skip_group_check=True,
                        )
                        nc.tensor.matmul(
                            po[:], mask, Rab[:, H : 2 * H],
                            start=False, stop=True, skip_group_check=True,
                        )
                        g, n = divmod(j, NSUB)
                        nc.scalar.copy(h2sup[g][:, n, :], ph[:])
                        nc.scalar.copy(outsup[g][:, n, :], po[:])
                        if n == NSUB - 1:
                            base = b * t_len + g * SUPER
                            nc.sync.dma_start(
                                out=hid_d[base : base + SUPER, :].rearrange(
                                    "(n p) h -> p n h", p=P
                                ),
                                in_=h2sup.pop(g)[:],
                            )
                            nc.sync.dma_start(
                                out=out_d[base : base + SUPER, :].rearrange(
                                    "(n p) h -> p n h", p=P
                                ),
                                in_=outsup.pop(g)[:],
                            )
                            xsup.pop(g, None)

                    # ---- stage 1b: cast yT to fp16 for the projections
                    if k < ngb:
                        yt = ytpool.tile([P, H], F16, name="yt")
                        nc.vector.tensor_copy(yt[:], pyts.pop(k)[:])
                        yts[k] = yt

                    # ---- stage 2 (one block behind): local projections
                    i = k - 1
                    if 0 <= i < ngb:
                        yt = yts.pop(i)
                        ph = pso.tile([P, H], F32, tag="ph", name="ph")
                        po = pso.tile([P, H], F32, tag="po", name="po")
                        for c in range(4):
                            lhs = yt[:, c * P : (c + 1) * P]
                            nc.tensor.matmul(
                                ph[:], lhs, wa_sb[:, c * H : (c + 1) * H],
                                start=(c == 0), stop=(c == 3),
                            )
                            nc.tensor.matmul(
                                po[:], lhs, wb_sb[:, c * H : (c + 1) * H],
                                start=(c == 0), stop=(c == 3),
                            )
                        phs[i], pos[i] = ph, po
                        # row extraction: T2U's last column is all-ones, so the
                        # projection's psum row 127 holds ba (the projected
                        # block sum, computed WITHOUT cancellation); row 126 is
                        # the true r126.  Store (r126, ba) raw -- the mask
                        # algebra reconstructs r127 = r126 + ba.  One two-lane
                        # cast-copy per output (GpSimd can't touch PSUM; engine
                        # SBUF writes must start at partition 0/32/64/96), then
                        # a tiny SBUF->SBUF DMA scatters into Rstack rows
                        # 2i:2i+2 (DMA has no partition alignment limits).
                        # (engine PSUM reads must also start at a quadrant
                        # boundary, so copy the whole last quadrant -- the 32
                        # lanes run in parallel, same duration as 2 rows --
                        # then one small DMA scatters both outputs' rows)
                        s2 = scrpool.tile([32, 2 * H], F16, tag="s2", name="s2")
                        nc.vector.tensor_copy(s2[:, 0:H], ph[96:128, :])
                        nc.scalar.copy(s2[:, H : 2 * H], po[96:128, :])
                        nc.sync.dma_start(
                            out=Rab[2 * i : 2 * i + 2, :], in_=s2[30:32, :]
                        )
    if not nc.is_finalized():
        nc.finalize()
    return nc


def make_consts(W_ih: np.ndarray, W_out: np.ndarray) -> dict[str, np.ndarray]:
    W0 = W_ih[0].astype(np.float64)
    W1 = W_ih[1].astype(np.float64)
    Wa64 = W1 @ W0
    Wb64 = W_out.astype(np.float64) @ Wa64

    # [i, o] chunked along i into 4 partition groups -> [128, 4*512]
    def pack_w(w64):
        wT = w64.T.astype(np.float16)  # [i, o]
        return np.ascontiguousarray(
            wT.reshape(4, P, H).transpose(1, 0, 2).reshape(P, 4 * H)
        )

    tau = np.arange(P, dtype=np.float32)
    s_idx = tau[:, None]
    t_idx = tau[None, :]

    cpack = np.zeros((P, C_TOT), dtype=np.float32)
    cpack[:, C_WA : C_WA + 4 * H] = pack_w(Wa64)
    cpack[:, C_WB : C_WB + 4 * H] = pack_w(Wb64)
    t2u = np.where(t_idx >= s_idx, t_idx - s_idx + 1.0, 0.0)
    # last column emits the raw block sum instead of y_loc[:,127] so the
    # projection produces ba (no catastrophic cancellation); the mask's
    # self-fix rows repair output row 127 afterwards
    t2u[:, P - 1] = 1.0
    cpack[:, C_T2U : C_T2U + P] = t2u
    # carry mask tiles.  Rstack rows: 2q = r126(q), 2q+1 = ba(q); the true
    # last row is r127 = r126 + ba, so
    #   carry(k, tau) = sum_{q<k} (128*(k-q)+tau-127)*ba(q) + r127(q)
    #                 = sum_{q<k} (128*(k-q)+tau-126)*ba(q) + r126(q)
    # plus a self-fix row for block k: output row 127 currently holds ba(k)
    # and must become r127(k) -> add r126(k) at tau=127 only.
    for k in range(NGB):
        col = C_MASK + k * P
        for q in range(k):
            d = float(k - q)
            cpack[2 * q, col : col + P] = 1.0
            cpack[2 * q + 1, col : col + P] = 128.0 * d + tau - 126.0
        cpack[2 * k, col + P - 1] = 1.0
    return {"cpack": cpack.astype(np.float16)}


def make_in_maps(x: np.ndarray, W_ih: np.ndarray, W_out: np.ndarray):
    consts = make_consts(np.asarray(W_ih, np.float32), np.asarray(W_out, np.float32))
    xs = (np.asarray(x, np.float32) * SCALE).astype(np.float16)
    in_maps = []
    for core in range(NCORES):
        shard = np.ascontiguousarray(
            xs[core * BPC : (core + 1) * BPC].reshape(BPC * T, H)
        )
        in_maps.append({"x": shard, **consts})
    return in_maps


def gather_outputs(results):
    outs = np.concatenate(
        [r["outputs"].reshape(BPC, T, H).astype(np.float32) for r in results],
        axis=0,
    ) * (1.0 / SCALE)
    hids = np.concatenate(
        [r["hidden"].reshape(BPC, T, H).astype(np.float32) for r in results],
        axis=0,
    ) * (1.0 / SCALE)
    return outs, hids


def kernel(x: np.ndarray, W_ih: np.ndarray, W_out: np.ndarray):
    nc = build_nc()
    in_maps = make_in_maps(x, W_ih, W_out)
    res = run_bass_kernel_spmd(nc, in_maps, core_ids=list(range(NCORES)))
    return gather_outputs(res.results)


# revision 22
# speedup vs baseline: 1.0814x; 1.0208x over previous
"""Trainium2 Bass kernel for a 2-layer linear RNN (identity state transition).

Math: the reference computes, per layer l, h = cumsum_t(h @ W_l^T) and then
outputs = h @ W_out^T.  Cumsum along time commutes with the (time-independent)
feature matmuls, so with Wa = W1 @ W0 and Wb = W_out @ Wa:

    hidden  = cumsum_t(cumsum_t(x)) @ Wa^T
    outputs = cumsum_t(cumsum_t(x)) @ Wb^T

The double cumsum y = C^2 x has closed form y[t] = sum_{s<=t} (t-s+1) x[s].
Blockwise (128-step blocks) it is computed on the PE in two parts:

  1. LOCAL: yT_chunk = matmul(lhsT=x_chunk, rhs=T2U) with the constant
     triangular T2U[s,t'] = (t'-s+1 for s<=t') gives [feature, time] chunks
     with no explicit PE transposes; yT is exactly the operand layout the two
     weight matmuls need as lhsT.  The local projections ph/po = yT.T @ W are
     carry-free.
  2. CARRY (applied POST-projection): the cross-block contribution of block q
     to block k at local time tau is (128*D + tau - 127)*ba(q) + r127(q) with
     D = k-q, where ba(q) (block sum of projected inputs) and r127(q) (last
     row) are linear functions of rows 126/127 of the LOCAL projection of
     block q.  Tiny per-block row extractions (GpSimd) append
     [ba; r127] to a persistent Rstack[64, 512]; the whole carry for block k
     is then ONE matmul per output: maskT_k[0:2k].T @ Rstack[0:2k] with a
     host-precomputed constant mask table.  (ba is stored explicitly --
     forming r127-r126 inside the matmul would amplify fp16 quantization by
     |r127|/|ba| ~ 70x.)

This removes all per-block carry-state matmuls from the v2 design.  A
software pipeline (projection runs one block behind the local cumsum,
correction one further behind) keeps the PE dense and hides the PSUM->SBUF
cast latencies.

Dtype strategy: everything on-device is float16 (inputs pre-scaled by 1/64 on
the host so the double-cumsum magnitudes stay inside fp16 range; outputs are
scaled back by 64 on the host).  fp16 matmuls run the PE at the full 2.4 GHz
warm clock with fast weight load, stream 1 column/cycle, and halve DMA and
on-chip copy traffic vs fp32.  PSUM accumulation stays fp32.  All constant
coefficient tables (integers <= 4096) are exact-to-half-ulp in fp16.

Sharding: data-parallel over batch, 2 of 16 batch elements per core, weights
replicated.
"""

import numpy as np

import concourse.bass as bass
import concourse.bacc as bacc
import concourse.mybir as mybir
from concourse.tile import TileContext
from concourse.bass_utils import run_bass_kernel_spmd

P = 128          # partitions / time-block size
H = 512          # hidden/input/output feature dim
T = 4096         # sequence length
B = 16           # batch
NCORES = 8
BPC = B // NCORES            # batch elements per core = 2
NSUB = 4                     # 128-step sub-tiles per super-tile
SUPER = P * NSUB             # 512 timesteps per DMA super-tile
NGB = T // P                 # 128-step blocks per batch element = 32

F32 = mybir.dt.float32
F16 = mybir.dt.float16

SCALE = 1.0 / 64.0           # host pre-scale keeping fp16 in range

# column offsets inside the packed fp16 constant block
C_WA = 0
C_WB = C_WA + 4 * H          # 2048
C_T2U = C_WB + 4 * H         # 4096
C_MASK = C_T2U + P           # 4224: 32 carry-mask tiles [64 x 128] side by side
C_TOT = C_MASK + NGB * P     # 8320


def build_nc(bpc: int = BPC, t_len: int = T) -> bass.Bass:
    ngb = t_len // P         # 128-step blocks per batch element
    nc = bacc.Bacc(None, target_bir_lowering=False)

    x_d = nc.dram_tensor("x", [bpc * t_len, H], F16, kind="ExternalInput")
    cpack_d = nc.dram_tensor("cpack", [P, C_TOT], F16, kind="ExternalInput")
    out_d = nc.dram_tensor("outputs", [bpc * t_len, H], F16, kind="ExternalOutput")
    hid_d = nc.dram_tensor("hidden", [bpc * t_len, H], F16, kind="ExternalOutput")

    with TileContext(nc) as tc:
        with (
            tc.tile_pool(name="consts", bufs=1) as cpool,
            tc.tile_pool(name="xs", bufs=3) as xpool,
            tc.tile_pool(name="staged", bufs=3) as stpool,
            tc.tile_pool(name="ytsb", bufs=3) as ytpool,
            tc.tile_pool(name="rstack", bufs=2) as rpool,
            tc.tile_pool(name="rscr", bufs=4) as scrpool,
            tc.tile_pool(name="psyt", bufs=2, space="PSUM") as psyt,
            tc.tile_pool(name="pso", bufs=3, space="PSUM") as pso,
        ):
            cpack = cpool.tile([P, C_TOT], F16)
            nc.sync.dma_start(out=cpack[:], in_=cpack_d[:])

            wa_sb = cpack[:, C_WA : C_WA + 4 * H]
            wb_sb = cpack[:, C_WB : C_WB + 4 * H]
            t2u_sb = cpack[:, C_T2U : C_T2U + P]

            for b in range(bpc):
                # rows 2q = r126(q), 2q+1 = ba(q); cols [0:H] for the hidden
                # projection, [H:2H] for the output projection.  Zeroed so the
                # constant-K=128 corr matmuls can read not-yet-written rows
                # (their mask coefficients are zero, but NaN*0 != 0).
                Rab = rpool.tile([P, 2 * H], F16, tag="Rab")
                nc.gpsimd.memset(Rab[:], 0.0)
                xsup = {}
                h2sup = {}
                outsup = {}
                pyts = {}
                yts = {}
                phs = {}
                pos = {}
                for k in range(ngb + 3):
                    # ---- stage 0: DMA in super-tile
                    if k < ngb and k % NSUB == 0:
                        g = k // NSUB
                        base = b * t_len + g * SUPER
                        xs = xpool.tile([P, NSUB, H], F16, name="xs")
                        nc.sync.dma_start(
                            out=xs[:],
                            in_=x_d[base : base + SUPER, :].rearrange(
                                "(n p) h -> p n h", p=P
                            ),
                        )
                        xsup[g] = xs
                        h2sup[g] = stpool.tile([P, NSUB, H], F16, tag="h2s", name="h2s")
                        outsup[g] = stpool.tile([P, NSUB, H], F16, tag="outs", name="outs")

                    # ---- stage 1: local double cumsum for block k
                    if k < ngb:
                        x_t = xsup[k // NSUB][:, k % NSUB, :]
                        pyt = psyt.tile([P, H], F32, name="pyt")
                        for c in range(4):
                            nc.tensor.matmul(
                                pyt[:, c * P : (c + 1) * P],
                                x_t[:, c * P : (c + 1) * P],
                                t2u_sb,
                                start=True, stop=True,
                            )
                        pyts[k] = pyt

                    # ---- stage 3 (three blocks behind): carry corr + copies
                    j = k - 3
                    if j >= 0:
                        ph, po = phs.pop(j), pos.pop(j)
                        # rows [0:2j] apply the cross-block carry; rows
                        # [2j:2j+2] repair this block's own row 127 (the
                        # projection emitted ba there, not r127).  K is padded
                        # to the full 128 (zero mask rows) to stay on the
                        # full-array matmul path.
                        mask = cpack[:, C_MASK + j * P : C_MASK + (j + 1) * P]
                        nc.tensor.matmul(
                            ph[:], mask, Rab[:, 0:H],
                            start=False, stop=True, skip_group_check=True,
                        )
                        nc.tensor.matmul(
                            po[:], mask, Rab[:, H : 2 * H],
                            start=False, stop=True, skip_group_check=True,
                        )
                        g, n = divmod(j, NSUB)
                        nc.scalar.copy(h2sup[g][:, n, :], ph[:])
                        nc.scalar.copy(outsup[g][:, n, :], po[:])
                        if n == NSUB - 1:
                            base = b * t_len + g * SUPER
                            nc.sync.dma_start(
                                out=hid_d[base : base + SUPER, :].rearrange(
                                    "(n p) h -> p n h", p=P
                                ),
                                in_=h2sup.pop(g)[:],
                            )
                            nc.sync.dma_start(
                                out=out_d[base : base + SUPER, :].rearrange(
                                    "(n p) h -> p n h", p=P
                                ),
                                in_=outsup.pop(g)[:],
                            )
                            xsup.pop(g, None)

                    # ---- stage 1b: cast yT to fp16 for the projections
                    if k < ngb:
                        yt = ytpool.tile([P, H], F16, name="yt")
                        nc.vector.tensor_copy(yt[:], pyts.pop(k)[:])
                        yts[k] = yt

                    # ---- stage 2b (two blocks behind): row extraction.
                    # Done a full iteration after the projections so the s2
                    # copies never head-of-line-block the casts in the
                    # engine queues (their deps completed last iteration).
                    i2 = k - 2
                    if 0 <= i2 < ngb:
                        ph2, po2 = phs[i2], pos[i2]
                        # T2U's last column is all-ones, so psum row 127 holds
                        # ba (projected block sum, no cancellation); row 126 is
                        # r126.  Engine PSUM reads must start at a quadrant
                        # boundary, so copy the whole last quadrant (32 lanes
                        # run in parallel, same duration as 2 rows), then one
                        # small DMA scatters both outputs' rows into Rab.
                        s2 = scrpool.tile([32, 2 * H], F16, tag="s2", name="s2")
                        nc.vector.tensor_copy(s2[:, 0:H], ph2[96:128, :])
                        nc.scalar.copy(s2[:, H : 2 * H], po2[96:128, :])
                        nc.sync.dma_start(
                            out=Rab[2 * i2 : 2 * i2 + 2, :], in_=s2[30:32, :]
                        )

                    # ---- stage 2 (one block behind): local projections
                    i = k - 1
                    if 0 <= i < ngb:
                        yt = yts.pop(i)
                        ph = pso.tile([P, H], F32, tag="ph", name="ph")
                        po = pso.tile([P, H], F32, tag="po", name="po")
                        for c in range(4):
                            lhs = yt[:, c * P : (c + 1) * P]
                            nc.tensor.matmul(
                                ph[:], lhs, wa_sb[:, c * H : (c + 1) * H],
                                start=(c == 0), stop=(c == 3),
                            )
                            nc.tensor.matmul(
                                po[:], lhs, wb_sb[:, c * H : (c + 1) * H],
                                start=(c == 0), stop=(c == 3),
                            )
                        phs[i], pos[i] = ph, po
    if not nc.is_finalized():
        nc.finalize()
    return nc


def make_consts(W_ih: np.ndarray, W_out: np.ndarray) -> dict[str, np.ndarray]:
    W0 = W_ih[0].astype(np.float64)
    W1 = W_ih[1].astype(np.float64)
    Wa64 = W1 @ W0
    Wb64 = W_out.astype(np.float64) @ Wa64

    # [i, o] chunked along i into 4 partition groups -> [128, 4*512]
    def pack_w(w64):
        wT = w64.T.astype(np.float16)  # [i, o]
        return np.ascontiguousarray(
            wT.reshape(4, P, H).transpose(1, 0, 2).reshape(P, 4 * H)
        )

    tau = np.arange(P, dtype=np.float32)
    s_idx = tau[:, None]
    t_idx = tau[None, :]

    cpack = np.zeros((P, C_TOT), dtype=np.float32)
    cpack[:, C_WA : C_WA + 4 * H] = pack_w(Wa64)
    cpack[:, C_WB : C_WB + 4 * H] = pack_w(Wb64)
    t2u = np.where(t_idx >= s_idx, t_idx - s_idx + 1.0, 0.0)
    # last column emits the raw block sum instead of y_loc[:,127] so the
    # projection produces ba (no catastrophic cancellation); the mask's
    # self-fix rows repair output row 127 afterwards
    t2u[:, P - 1] = 1.0
    cpack[:, C_T2U : C_T2U + P] = t2u
    # carry mask tiles.  Rstack rows: 2q = r126(q), 2q+1 = ba(q); the true
    # last row is r127 = r126 + ba, so
    #   carry(k, tau) = sum_{q<k} (128*(k-q)+tau-127)*ba(q) + r127(q)
    #                 = sum_{q<k} (128*(k-q)+tau-126)*ba(q) + r126(q)
    # plus a self-fix row for block k: output row 127 currently holds ba(k)
    # and must become r127(k) -> add r126(k) at tau=127 only.
    for k in range(NGB):
        col = C_MASK + k * P
        for q in range(k):
            d = float(k - q)
            cpack[2 * q, col : col + P] = 1.0
            cpack[2 * q + 1, col : col + P] = 128.0 * d + tau - 126.0
        cpack[2 * k, col + P - 1] = 1.0
    return {"cpack": cpack.astype(np.float16)}


def make_in_maps(x: np.ndarray, W_ih: np.ndarray, W_out: np.ndarray):
    consts = make_consts(np.asarray(W_ih, np.float32), np.asarray(W_out, np.float32))
    xs = (np.asarray(x, np.float32) * SCALE).astype(np.float16)
    in_maps = []
    for core in range(NCORES):
        shard = np.ascontiguousarray(
            xs[core * BPC : (core + 1) * BPC].reshape(BPC * T, H)
        )
        in_maps.append({"x": shard, **consts})
    return in_maps


def gather_outputs(results):
    outs = np.concatenate(
        [r["outputs"].reshape(BPC, T, H).astype(np.float32) for r in results],
        axis=0,
    ) * (1.0 / SCALE)
    hids = np.concatenate(
        [r["hidden"].reshape(BPC, T, H).astype(np.float32) for r in results],
        axis=0,
    ) * (1.0 / SCALE)
    return outs, hids


def kernel(x: np.ndarray, W_ih: np.ndarray, W_out: np.ndarray):
    nc = build_nc()
    in_maps = make_in_maps(x, W_ih, W_out)
    res = run_bass_kernel_spmd(nc, in_maps, core_ids=list(range(NCORES)))
    return gather_outputs(res.results)
